# revision 3
# baseline (speedup 1.0000x reference)
"""Bi-directional correlation cost volume on 8 Trainium2 NeuronCores.

Strategy (data-parallel over batch, one batch element per core):
  - Per core, compute the Gram band G[u, x] = sum_c L[c,h,u] * R[c,h,x] / C
    for |x - u| <= 63 with TensorE matmuls (K=C=32, 4x row-tiled over
    h-groups so 4 matmuls share the PE array).
  - Stage the band rectangles to HBM as [h, chunk, u, x-window].
  - The cost volume out[d, x] = G[x -/+ d, x] is a *shear* of the band;
    host extracts the 127 diagonals with one vectorized gather per batch.
"""

import numpy as np

B, C, H, WIMG, D = 8, 32, 160, 320, 64
# (u0, U, xw0, W): u-chunk start/size, x-window start/size
CHUNKS = [(0, 128, 0, 191), (128, 128, 65, 254), (256, 64, 193, 127)]
WSLOT = 256
HQ = H // 4  # h-rows per PE quadrant

_CACHE = {}


def _get_nc():
    if "nc" in _CACHE:
        return _CACHE["nc"]
    import concourse.bacc as bacc
    import concourse.tile as tile
    from concourse import mybir

    f32 = mybir.dt.float32
    nc = bacc.Bacc("TRN2", target_bir_lowering=False, debug=False)
    r_in = nc.declare_dram_parameter("r_in", [C, H, WIMG], f32, isOutput=False)
    l_in = nc.declare_dram_parameter("l_in", [C, H, WIMG], f32, isOutput=False)
    stag = nc.declare_dram_parameter("stag", [H, 3, 128, WSLOT], f32, isOutput=True)

    with tile.TileContext(nc) as tc:
        with tc.tile_pool(name="inp", bufs=1) as inp_pool, \
             tc.tile_pool(name="ps", bufs=6, space="PSUM") as ps_pool, \
             tc.tile_pool(name="st", bufs=8) as st_pool:
            Lsb = inp_pool.tile([128, HQ * WIMG], f32, tag="L")
            Rsb = inp_pool.tile([128, HQ * WIMG], f32, tag="R")
            # partition (q, c) holds h-rows [40q, 40q+40) of channel c
            for q in range(4):
                nc.sync.dma_start(
                    Lsb[32 * q:32 * (q + 1), :],
                    l_in[:, HQ * q:HQ * (q + 1), :].rearrange(
                        "c hh x -> c (hh x)"),
                )
                nc.sync.dma_start(
                    Rsb[32 * q:32 * (q + 1), :],
                    r_in[:, HQ * q:HQ * (q + 1), :].rearrange(
                        "c hh x -> c (hh x)"),
                )
            for hh in range(HQ):
                for q in range(4):
                    h = HQ * q + hh
                    for ci, (u0, U, xw0, W) in enumerate(CHUNKS):
                        ps = ps_pool.tile([128, 256], f32, tag="ps")
                        nc.tensor.matmul(
                            ps[:U, :W],
                            Lsb[32 * q:32 * (q + 1),
                                hh * WIMG + u0:hh * WIMG + u0 + U],
                            Rsb[32 * q:32 * (q + 1),
                                hh * WIMG + xw0:hh * WIMG + xw0 + W],
                            start=True, stop=True,
                            tile_position=(32 * q, 0),
                        )
                        sb = st_pool.tile([128, 256], f32, tag="sb")
                        if (hh + q) % 2:
                            nc.vector.tensor_scalar_mul(
                                sb[:U, :W], ps[:U, :W], 1.0 / C)
                        else:
                            nc.scalar.mul(sb[:U, :W], ps[:U, :W], 1.0 / C)
                        nc.sync.dma_start(stag[h, ci, :U, :W], sb[:U, :W])
    nc.compile()
    _CACHE["nc"] = nc
    return nc


def _gather_idx():
    if "idx" in _CACHE:
        return _CACHE["idx"]
    P_ = np.arange(2 * D)[:, None]
    dts = np.where(P_ < D, P_, -(P_ - D))  # signed disparity per output plane
    x = np.arange(WIMG)[None, :]
    u = np.clip(x - dts, 0, WIMG - 1)
    c = np.minimum(u // 128, 2)
    u0 = c * 128
    xw0 = np.choose(c, [ch[2] for ch in CHUNKS])
    idx2d = ((c * 128) + (u - u0)) * WSLOT + np.clip(x - xw0, 0, WSLOT - 1)
    _CACHE["idx"] = np.ascontiguousarray(idx2d.reshape(-1).astype(np.int64))
    return _CACHE["idx"]


def _assemble(stag_b):
    """stag_b: [H, 3, 128, WSLOT] -> out_b [2D, H, WIMG]"""
    idx = _gather_idx()
    flat = stag_b.reshape(H, -1)
    o = np.empty((H, 2 * D, WIMG), dtype=np.float32)
    ov = o.reshape(H, -1)
    for h in range(H):
        np.take(flat[h], idx, out=ov[h])
    o = np.ascontiguousarray(o.transpose(1, 0, 2))
    for d in range(1, D):
        o[d, :, :d] = 0
        o[D + d, :, WIMG - d:] = 0
    return o


def run_cores(right_np, left_np, timing_reps=0):
    """Run the SPMD bass kernel; returns (list of staging arrays, exec_ns)."""
    from concourse.bass_utils import run_bass_kernel_spmd

    nc = _get_nc()
    in_maps = [
        {"r_in": np.ascontiguousarray(right_np[b]),
         "l_in": np.ascontiguousarray(left_np[b])}
        for b in range(B)
    ]
    res = run_bass_kernel_spmd(nc, in_maps, list(range(B)))
    return [res.results[b]["stag"] for b in range(B)]


def kernel(right_feature, left_feature, max_disp):
    assert int(max_disp) == D
    right_np = np.asarray(right_feature, dtype=np.float32)
    left_np = np.asarray(left_feature, dtype=np.float32)
    stags = run_cores(right_np, left_np)
    out = np.stack([_assemble(s) for s in stags])
    return out


# revision 11
# speedup vs baseline: 1.1713x; 1.1713x over previous
"""Bi-directional correlation cost volume on 8 Trainium2 NeuronCores.

Strategy (data-parallel over batch, one batch element per core):
  - Per core, compute the Gram band G[u, x] = sum_c L[c,h,u] * R[c,h,x] / C
    for |x - u| <= 63 with TensorE matmuls (K=C=32, 4x row-tiled over
    h-groups so 4 matmuls share the PE array).
  - Stage the band rectangles to HBM as [h, chunk, u, x-window].
  - The cost volume out[d, x] = G[x -/+ d, x] is a *shear* of the band;
    host extracts the 127 diagonals with one vectorized gather per batch.
"""

import numpy as np

B, C, H, WIMG, D = 8, 32, 160, 320, 64
# (u0, U, xw0, W): u-chunk start/size, x-window start/size
CHUNKS = [(0, 128, 0, 191), (128, 128, 65, 254), (256, 64, 193, 127)]
WSLOT = 256
HQ = H // 4  # h-rows per PE quadrant

_CACHE = {}


HGRP = 16      # h-rows batched per store DMA
ACT_MOD = 3    # every ACT_MOD-th copy goes to ScalarE (0 = all DVE)
STAG_BF16 = False  # stage the Gram band in bf16 (halves store traffic)


def _get_nc(reps=1):
    key = ("nc", reps, HGRP, ACT_MOD, STAG_BF16)
    if key in _CACHE:
        return _CACHE[key]
    import concourse.bacc as bacc
    import concourse.tile as tile
    from concourse import mybir

    f32 = mybir.dt.float32
    sdt = mybir.dt.bfloat16 if STAG_BF16 else f32
    nc = bacc.Bacc("TRN2", target_bir_lowering=False, debug=False)
    r_in = nc.declare_dram_parameter("r_in", [C, H, WIMG], f32, isOutput=False)
    l_in = nc.declare_dram_parameter("l_in", [C, H, WIMG], f32, isOutput=False)
    stag = nc.declare_dram_parameter("stag", [H, 3, 128, WSLOT], sdt, isOutput=True)

    with tile.TileContext(nc) as tc:
        with tc.tile_pool(name="inp", bufs=1) as inp_pool, \
             tc.tile_pool(name="ps", bufs=6, space="PSUM") as ps_pool, \
             tc.tile_pool(name="st", bufs=6) as st_pool:
            Lsb = inp_pool.tile([128, HQ * WIMG], f32, tag="L")
            Rsb = inp_pool.tile([128, HQ * WIMG], f32, tag="R")
            # partition (q, c) holds h-rows [40q, 40q+40) of channel c
            for q in range(4):
                nc.sync.dma_start(
                    Lsb[32 * q:32 * (q + 1), :],
                    l_in[:, HQ * q:HQ * (q + 1), :].rearrange(
                        "c hh x -> c (hh x)"),
                )
                nc.sync.dma_start(
                    Rsb[32 * q:32 * (q + 1), :],
                    r_in[:, HQ * q:HQ * (q + 1), :].rearrange(
                        "c hh x -> c (hh x)"),
                )
            for _ in range(reps):
                for q in range(4):
                    for hh0 in range(0, HQ, HGRP):
                        G = min(HGRP, HQ - hh0)
                        for ci, (u0, U, xw0, W) in enumerate(CHUNKS):
                            sb = st_pool.tile([128, HGRP * WSLOT], sdt,
                                              tag="sb")
                            for g in range(G):
                                hh = hh0 + g
                                ps = ps_pool.tile([128, 256], f32, tag="ps")
                                nc.tensor.matmul(
                                    ps[:U, :W],
                                    Lsb[32 * q:32 * (q + 1),
                                        hh * WIMG + u0:hh * WIMG + u0 + U],
                                    Rsb[32 * q:32 * (q + 1),
                                        hh * WIMG + xw0:hh * WIMG + xw0 + W],
                                    start=True, stop=True,
                                    tile_position=(32 * q, 0),
                                )
                                dst = sb[:U, g * WSLOT:g * WSLOT + W]
                                if ACT_MOD and hh % ACT_MOD == ACT_MOD - 1:
                                    nc.scalar.mul(dst, ps[:U, :W], 1.0 / C)
                                else:
                                    nc.vector.tensor_scalar_mul(
                                        dst, ps[:U, :W], 1.0 / C)
                            h0 = HQ * q + hh0
                            dma_eng = nc.sync if ci % 2 else nc.scalar
                            dma_eng.dma_start(
                                stag[h0:h0 + G, ci, :U, :W].rearrange(
                                    "g u w -> u g w"),
                                sb[:U, :].rearrange(
                                    "u (g w) -> u g w", g=HGRP)[:, :G, :W],
                            )
    nc.compile()
    _CACHE[key] = nc
    return nc


def _gather_idx():
    if "idx" in _CACHE:
        return _CACHE["idx"]
    P_ = np.arange(2 * D)[:, None]
    dts = np.where(P_ < D, P_, -(P_ - D))  # signed disparity per output plane
    x = np.arange(WIMG)[None, :]
    u = np.clip(x - dts, 0, WIMG - 1)
    c = np.minimum(u // 128, 2)
    u0 = c * 128
    xw0 = np.choose(c, [ch[2] for ch in CHUNKS])
    idx2d = ((c * 128) + (u - u0)) * WSLOT + np.clip(x - xw0, 0, WSLOT - 1)
    _CACHE["idx"] = np.ascontiguousarray(idx2d.reshape(-1).astype(np.int64))
    return _CACHE["idx"]


def _assemble(stag_b):
    """stag_b: [H, 3, 128, WSLOT] -> out_b [2D, H, WIMG]"""
    idx = _gather_idx()
    flat = np.asarray(stag_b).astype(np.float32).reshape(H, -1)
    o = np.empty((H, 2 * D, WIMG), dtype=np.float32)
    ov = o.reshape(H, -1)
    for h in range(H):
        np.take(flat[h], idx, out=ov[h])
    o = np.ascontiguousarray(o.transpose(1, 0, 2))
    for d in range(1, D):
        o[d, :, :d] = 0
        o[D + d, :, WIMG - d:] = 0
    return o


def run_cores(right_np, left_np, timing_reps=0):
    """Run the SPMD bass kernel; returns (list of staging arrays, exec_ns)."""
    from concourse.bass_utils import run_bass_kernel_spmd

    nc = _get_nc()
    in_maps = [
        {"r_in": np.ascontiguousarray(right_np[b]),
         "l_in": np.ascontiguousarray(left_np[b])}
        for b in range(B)
    ]
    res = run_bass_kernel_spmd(nc, in_maps, list(range(B)))
    return [res.results[b]["stag"] for b in range(B)]


def kernel(right_feature, left_feature, max_disp):
    assert int(max_disp) == D
    right_np = np.asarray(right_feature, dtype=np.float32)
    left_np = np.asarray(left_feature, dtype=np.float32)
    stags = run_cores(right_np, left_np)
    out = np.stack([_assemble(s) for s in stags])
    return out


# revision 13
# speedup vs baseline: 271.2630x; 231.5818x over previous
"""Bi-directional correlation cost volume on 8 Trainium2 NeuronCores.

Strategy (data-parallel over batch, one batch element per core):
  - Per core, compute the Gram band G[u, x] = sum_c L[c,h,u] * R[c,h,x] / C
    for |x - u| <= 63 with TensorE matmuls (K=C=32, 4x row-tiled over
    h-groups so 4 matmuls share the PE array).
  - Stage the band rectangles to HBM as [h, chunk, u, x-window].
  - The cost volume out[d, x] = G[x -/+ d, x] is a *shear* of the band;
    host extracts the 127 diagonals with one vectorized gather per batch.
"""

import numpy as np

B, C, H, WIMG, D = 8, 32, 160, 320, 64
# (u0, U, xw0, W): u-chunk start/size, x-window start/size
CHUNKS = [(0, 128, 0, 191), (128, 128, 65, 254), (256, 64, 193, 127)]
WSLOT = 256
# packed staging: per h, chunk ci starts at COFF[ci]; row u is WPAD[ci] wide
# (64B-aligned) with the first W elements valid
WPAD = [192, 256, 128]
COFF = [0, 128 * 192, 128 * 192 + 128 * 256]
HROW = COFF[2] + 64 * 128  # 65536 elems per h-row
HQ = H // 4  # h-rows per PE quadrant

_CACHE = {}


HGRP = 16      # h-rows batched per store DMA
ACT_MOD = 3    # every ACT_MOD-th copy goes to ScalarE (0 = all DVE)
STAG_BF16 = False  # stage the Gram band in bf16 (halves store traffic)


def _get_nc(reps=1):
    key = ("nc", reps, HGRP, ACT_MOD, STAG_BF16)
    if key in _CACHE:
        return _CACHE[key]
    import concourse.bacc as bacc
    import concourse.tile as tile
    from concourse import mybir

    f32 = mybir.dt.float32
    sdt = mybir.dt.bfloat16 if STAG_BF16 else f32
    nc = bacc.Bacc("TRN2", target_bir_lowering=False, debug=False)
    r_in = nc.declare_dram_parameter("r_in", [C, H, WIMG], f32, isOutput=False)
    l_in = nc.declare_dram_parameter("l_in", [C, H, WIMG], f32, isOutput=False)
    stag = nc.declare_dram_parameter("stag", [H, HROW], sdt, isOutput=True)

    with tile.TileContext(nc) as tc:
        with tc.tile_pool(name="inp", bufs=1) as inp_pool, \
             tc.tile_pool(name="ps", bufs=6, space="PSUM") as ps_pool, \
             tc.tile_pool(name="st", bufs=6) as st_pool:
            Lsb = inp_pool.tile([128, HQ * WIMG], f32, tag="L")
            Rsb = inp_pool.tile([128, HQ * WIMG], f32, tag="R")
            # partition (q, c) holds h-rows [40q, 40q+40) of channel c
            for q in range(4):
                nc.sync.dma_start(
                    Lsb[32 * q:32 * (q + 1), :],
                    l_in[:, HQ * q:HQ * (q + 1), :].rearrange(
                        "c hh x -> c (hh x)"),
                )
                nc.sync.dma_start(
                    Rsb[32 * q:32 * (q + 1), :],
                    r_in[:, HQ * q:HQ * (q + 1), :].rearrange(
                        "c hh x -> c (hh x)"),
                )
            for _ in range(reps):
                for q in range(4):
                    for hh0 in range(0, HQ, HGRP):
                        G = min(HGRP, HQ - hh0)
                        for ci, (u0, U, xw0, W) in enumerate(CHUNKS):
                            sb = st_pool.tile([128, HGRP * WSLOT], sdt,
                                              tag="sb")
                            for g in range(G):
                                hh = hh0 + g
                                ps = ps_pool.tile([128, 256], f32, tag="ps")
                                nc.tensor.matmul(
                                    ps[:U, :W],
                                    Lsb[32 * q:32 * (q + 1),
                                        hh * WIMG + u0:hh * WIMG + u0 + U],
                                    Rsb[32 * q:32 * (q + 1),
                                        hh * WIMG + xw0:hh * WIMG + xw0 + W],
                                    start=True, stop=True,
                                    tile_position=(32 * q, 0),
                                )
                                dst = sb[:U, g * WSLOT:g * WSLOT + W]
                                if ACT_MOD and hh % ACT_MOD == ACT_MOD - 1:
                                    nc.scalar.mul(dst, ps[:U, :W], 1.0 / C)
                                else:
                                    nc.vector.tensor_scalar_mul(
                                        dst, ps[:U, :W], 1.0 / C)
                            h0 = HQ * q + hh0
                            dma_eng = nc.sync if ci % 2 else nc.scalar
                            dst_ap = stag[h0:h0 + G,
                                          COFF[ci]:COFF[ci] + U * WPAD[ci]]
                            dma_eng.dma_start(
                                dst_ap.rearrange(
                                    "g (u w) -> u g w", u=U)[:, :, :W],
                                sb[:U, :].rearrange(
                                    "u (g w) -> u g w", g=HGRP)[:, :G, :W],
                            )
    nc.compile()
    _CACHE[key] = nc
    return nc


def _gather_idx():
    if "idx" in _CACHE:
        return _CACHE["idx"]
    P_ = np.arange(2 * D)[:, None]
    dts = np.where(P_ < D, P_, -(P_ - D))  # signed disparity per output plane
    x = np.arange(WIMG)[None, :]
    u = np.clip(x - dts, 0, WIMG - 1)
    c = np.minimum(u // 128, 2)
    u0 = c * 128
    xw0 = np.choose(c, [ch[2] for ch in CHUNKS])
    Wc = np.choose(c, [ch[3] for ch in CHUNKS])
    wp = np.choose(c, WPAD)
    off = np.choose(c, COFF)
    w = np.clip(x - xw0, 0, Wc - 1)
    idx2d = off + (u - u0) * wp + w
    _CACHE["idx"] = np.ascontiguousarray(idx2d.reshape(-1).astype(np.int64))
    return _CACHE["idx"]


def _assemble(stag_b):
    """stag_b: [H, HROW] packed band -> out_b [2D, H, WIMG]"""
    idx = _gather_idx()
    flat = np.asarray(stag_b).astype(np.float32).reshape(H, -1)
    o = np.empty((H, 2 * D, WIMG), dtype=np.float32)
    ov = o.reshape(H, -1)
    for h in range(H):
        np.take(flat[h], idx, out=ov[h])
    o = np.ascontiguousarray(o.transpose(1, 0, 2))
    for d in range(1, D):
        o[d, :, :d] = 0
        o[D + d, :, WIMG - d:] = 0
    return o


def run_cores(right_np, left_np, timing_reps=0):
    """Run the SPMD bass kernel; returns (list of staging arrays, exec_ns)."""
    from concourse.bass_utils import run_bass_kernel_spmd

    nc = _get_nc()
    in_maps = [
        {"r_in": np.ascontiguousarray(right_np[b]),
         "l_in": np.ascontiguousarray(left_np[b])}
        for b in range(B)
    ]
    res = run_bass_kernel_spmd(nc, in_maps, list(range(B)))
    return [res.results[b]["stag"] for b in range(B)]


def kernel(right_feature, left_feature, max_disp):
    assert int(max_disp) == D
    right_np = np.asarray(right_feature, dtype=np.float32)
    left_np = np.asarray(left_feature, dtype=np.float32)
    stags = run_cores(right_np, left_np)
    out = np.stack([_assemble(s) for s in stags])
    return out
